# revision 2
# baseline (speedup 1.0000x reference)
# Trainium2 Bass kernel for nn_EnergyInGraph (espaloma-style graph energy sum).
#
# Math:
#   u2 = 0.5*k2*(x2-eq2)^2            [N2, C]  harmonic bonds
#   u3 = 0.5*k3*(x3-eq3)^2            [N3, C]  harmonic angles
#   u4 = sum_p k4_p*(1+cos(p*x4))     [N4, C]  periodic torsions (phases=0)
#   out[g, c] = segment_sum(u2)+segment_sum(u3)+segment_sum(u4)   [G, C]
#
# Strategy (data-parallel over graphs, 8 cores, one SPMD program):
#   Host sorts each stream by gid, assigns 512 graphs per core, and packs
#   nodes into fixed gid-width windows of exactly 32 blocks (<=128 rows,
#   <=4 consecutive gids per block).  Window widths / sel offsets are tuned
#   to the actual data (max over cores) so the single program fits all
#   cores.
#   L1: per block, narrow-stationary matmuls ([128,4] one-hot / weighted
#   one-hot vs [128,50] basis) accumulate [4,50] strips into a 2-bank PSUM
#   tile (partition bases 0/64, 16 free slots -> 32 blocks).
#   L2: PSUM tile is copied to SBUF f16 (GPSIMD), then one K=68 transposed
#   matmul per block PAIR (lhsT = hi[0:68, s, 0:50] data, rhs = fp8 sel
#   one-hot [68, W_SEL]) accumulates directly into the final transposed
#   PSUM accumulator acc[50, 512] at window-fixed free offsets.
#   Torsion basis: ACT sin for cos(px), p in {1,2,4,5}; DVE fused ops for
#   h3 = cos(3x)/2 = (c2-0.5)*c1 and b6 = 1+cos(6x) = 8*h3^2 (weight
#   factors folded on host: w'_3 = 2*w3; p=6 const excluded from u_const).

import os
import numpy as np
import ml_dtypes

import concourse.bacc as bacc
import concourse.tile as tile
from concourse import mybir
from concourse.bass_utils import run_bass_kernel_spmd

FP8 = ml_dtypes.float8_e4m3
F16 = np.float16

N2, N3, N4, C, PP, G = 200_000, 400_000, 300_000, 50, 6, 4096
NCORES = 8
GPC = G // NCORES          # graphs per core (512)
SPAN = 4                   # gid slots per block
BPW = 32                   # blocks per window (= one 2-bank PSUM L1 tile)
HALF_PI = float(np.pi / 2.0)

LAST_RESULTS = None        # BassKernelResults of the most recent run


# ----------------------------------------------------------------------------
# host-side packing
# ----------------------------------------------------------------------------

def _pack_window(gids, lo, hi):
    """Greedy blocks (<=128 rows, gid span < SPAN) for one window's
    gid-sorted slice. gids are window-local-sorted global-local gids.
    Returns list of (start, nrows, g0)."""
    blocks = []
    n = len(gids)
    i = 0
    while i < n:
        g0 = int(gids[i])
        j_max = min(i + 128, n)
        j = i + int(np.searchsorted(gids[i:j_max], g0 + SPAN, side="left"))
        smax = int(gids[j - 1]) - g0           # max used slot index
        blocks.append((i, j - i, g0, smax))
        i = j
    return blocks


def _core_blocks(gid_sorted, W):
    """Split one core's sorted gid stream into ceil(512/W) windows and pack
    blocks. Returns list over windows of block lists."""
    nw = (GPC + W - 1) // W
    bounds = np.searchsorted(gid_sorted, np.arange(0, (nw + 1) * W, W))
    wins = []
    for k in range(nw):
        s, e = bounds[k], bounds[k + 1]
        wins.append([(s + b0, nr, g0, smax) for (b0, nr, g0, smax)
                     in _pack_window(gid_sorted[s:e], k * W, min((k + 1) * W, GPC))])
    return wins


def _choose_W(streams_gids, W_hint):
    """Largest W (block window gid width) such that every window on every
    core packs into <= BPW blocks."""
    for W in range(W_hint, 3, -1):
        ok = True
        for gs in streams_gids:
            for blks in _core_blocks(gs, W):
                if len(blks) > BPW:
                    ok = False
                    break
            if not ok:
                break
        if ok:
            return W
    raise RuntimeError("no feasible window width")


def _sel_layout(all_wins, W, W_SEL0):
    """Choose W_SEL and per-(window k, pair s) absolute output bases, uniform
    across cores, covering every real block pair. all_wins: [core][k] ->
    blocks list. Returns (W_SEL, BASE[k][s])."""
    nw = max(len(w) for w in all_wins)
    W_SEL = W_SEL0
    while W_SEL <= 64:
        # required coverage per (k, s): min/max slot gid over cores
        ok = True
        BASE = np.zeros((nw, BPW // 2), np.int64)
        for k in range(nw):
            for s in range(BPW // 2):
                lo_need, hi_need = None, None
                for wins in all_wins:
                    if k >= len(wins):
                        continue
                    for bb in (2 * s, 2 * s + 1):
                        if bb < len(wins[k]):
                            g0, smax = wins[k][bb][2], wins[k][bb][3]
                            lo_need = g0 if lo_need is None else min(lo_need, g0)
                            hi_need = (g0 + smax + 1) if hi_need is None else max(hi_need, g0 + smax + 1)
                if lo_need is None:
                    BASE[k][s] = min(k * W, GPC - W_SEL)
                    continue
                if hi_need - lo_need > W_SEL:
                    ok = False
                    break
                base = min(max(lo_need, 0), GPC - W_SEL)
                if hi_need > base + W_SEL:
                    base = min(hi_need - W_SEL, GPC - W_SEL)
                if lo_need < base:
                    ok = False
                    break
                BASE[k][s] = base
            if not ok:
                break
        if ok:
            return W_SEL, BASE
        W_SEL += 4
    raise RuntimeError("sel width explosion")


def _prep_stream(x_sorted, gid_sorted, w_sorted, W):
    """Per-core block packing + device arrays for one stream.
    x_sorted:[R,C] f32; w_sorted [R,K] weights or None (harmonic).
    Returns (wins, arrays dict building callback inputs)."""
    wins = _core_blocks(gid_sorted, W)
    return wins


def _fill_arrays(wins, x_sorted, gid_sorted, w_sorted, NW, W_SEL, BASE):
    """Build xpack [128, NW*BPW*C] f32 (or xh/oh merged for harmonic),
    at [128, NW*BPW*PP*SPAN] f16 (torsion) and sel [128, NW*16*W_SEL] fp8."""
    torsion = w_sorted is not None
    B = NW * BPW
    xp = np.zeros((B, 128, C), np.float32)
    at = np.zeros((B, 128, PP, SPAN), np.float32) if torsion else None
    oh = np.zeros((B, 128, SPAN), np.float32) if not torsion else None
    sel = np.zeros((NW, BPW // 2, 128, W_SEL), np.float32)
    R = len(gid_sorted)
    for k in range(len(wins)):
        for bb, (b0, nr, g0, smax) in enumerate(wins[k]):
            bi = k * BPW + bb
            idx = b0 + np.arange(nr)
            xp[bi, :nr] = x_sorted[idx]
            slots = gid_sorted[idx] - g0            # [nr] in [0, SPAN)
            if torsion:
                at[bi, np.arange(nr)[:, None], np.arange(PP)[None, :],
                   slots[:, None]] = w_sorted[idx]
            else:
                oh[bi, np.arange(nr), slots] = 1.0
            # sel: pair s = bb//2, partial row = 64*(bb%2) + i
            s = bb // 2
            base = BASE[k][s]
            for i in range(smax + 1):
                col = g0 + i - base
                assert 0 <= col < W_SEL, (k, bb, g0, base)
                sel[k, s, 64 * (bb % 2) + i, col] = 1.0
    out = {
        "sel": np.ascontiguousarray(
            sel.transpose(2, 0, 1, 3).reshape(128, NW * (BPW // 2) * W_SEL)
        ).astype(FP8),
    }
    if torsion:
        out["xt"] = np.ascontiguousarray(
            xp.transpose(1, 0, 2).reshape(128, B * C))
        out["at"] = np.ascontiguousarray(
            at.transpose(1, 0, 2, 3).reshape(128, B * PP * SPAN)).astype(F16)
    else:
        xo = np.concatenate([xp, oh], axis=2)      # [B, 128, C+SPAN]
        out["xh"] = np.ascontiguousarray(
            xo.transpose(1, 0, 2).reshape(128, B * (C + SPAN))).astype(F16)
    return out


def _prep_host(x2, k2, eq2, gid2, x3, k3, eq3, gid3, x4, k4, phases4,
               periodicity4, gid4, n_graphs):
    G_ = int(n_graphs)
    assert G_ == G
    if np.count_nonzero(np.asarray(phases4)) != 0:
        raise NotImplementedError("nonzero torsion phases not supported")
    per = np.asarray(periodicity4)
    peri = np.rint(per).astype(np.int64)
    assert np.all((peri >= 1) & (peri <= PP))

    # torsion basis weights w4[n, p-1] = sum of k4 slots with periodicity p
    if np.array_equal(peri[0], np.arange(1, PP + 1)) and np.all(peri == peri[0]):
        w4 = np.asarray(k4, np.float32).copy()
    else:
        w4 = np.zeros((N4, PP), np.float32)
        np.add.at(w4, (np.arange(N4)[:, None], peri - 1), np.asarray(k4))

    # constant torsion term: sum_{p<=5} w_p per node -> per graph
    # (p=6 basis is 1+cos6 so its +1 is on device)
    const4 = np.asarray(w4[:, 0:5], np.float64).sum(1)
    u_const = np.bincount(np.asarray(gid4), weights=const4, minlength=G)
    # device basis for p=3 is cos(3x)/2 -> fold factor 2 into the weight;
    # p=6 basis is h3^2 = (1+cos 6x)/8 -> fold factor 8 into the weight
    w4[:, 2] *= 2.0
    w4[:, 5] *= 8.0

    # harmonic: fold scales into x on host: xh = sqrt(0.5 k) * (x - eq)
    s2 = np.sqrt(0.5 * np.asarray(k2, np.float32))
    s3 = np.sqrt(0.5 * np.asarray(k3, np.float32))
    xh = np.concatenate([(np.asarray(x2) - np.asarray(eq2)) * s2,
                         (np.asarray(x3) - np.asarray(eq3)) * s3],
                        0).astype(np.float32)
    gidh = np.concatenate([np.asarray(gid2), np.asarray(gid3)]).astype(np.int64)
    x4f = np.asarray(x4, np.float32)
    gid4l = np.asarray(gid4, np.int64)

    # sort streams by gid
    o = np.argsort(gidh, kind="stable")
    xh, gidh = xh[o], gidh[o]
    o = np.argsort(gid4l, kind="stable")
    x4s, gid4s, w4s = x4f[o], gid4l[o], w4[o]

    hsplit = np.searchsorted(gidh, np.arange(0, G + 1, GPC))
    tsplit = np.searchsorted(gid4s, np.arange(0, G + 1, GPC))
    t_g = [gid4s[tsplit[c]:tsplit[c + 1]] - c * GPC for c in range(NCORES)]
    h_g = [gidh[hsplit[c]:hsplit[c + 1]] - c * GPC for c in range(NCORES)]

    Wt = _choose_W(t_g, 60)
    Wh = _choose_W(h_g, 30)
    t_wins = [_core_blocks(g, Wt) for g in t_g]
    h_wins = [_core_blocks(g, Wh) for g in h_g]
    NWt = max(len(w) for w in t_wins)
    NWh = max(len(w) for w in h_wins)
    WSt, BASEt = _sel_layout(t_wins, Wt, 12)
    WSh, BASEh = _sel_layout(h_wins, Wh, 12)

    in_maps = []
    for c in range(NCORES):
        ts, te = tsplit[c], tsplit[c + 1]
        hs, he = hsplit[c], hsplit[c + 1]
        dt = _fill_arrays(t_wins[c], x4s[ts:te], t_g[c], w4s[ts:te],
                          NWt, WSt, BASEt)
        dh = _fill_arrays(h_wins[c], xh[hs:he], h_g[c], None,
                          NWh, WSh, BASEh)
        in_maps.append({"xt": dt["xt"], "at": dt["at"], "selt": dt["sel"],
                        "xh": dh["xh"], "selh": dh["sel"]})
    cfg = dict(NWt=NWt, NWh=NWh, WSt=WSt, WSh=WSh,
               BASEt=BASEt, BASEh=BASEh)
    return in_maps, cfg, u_const


# ----------------------------------------------------------------------------
# device kernel
# ----------------------------------------------------------------------------

def _build_nc(cfg):
    f32, f16, f8 = mybir.dt.float32, mybir.dt.float16, mybir.dt.float8e4
    SIN = mybir.ActivationFunctionType.Sin
    ADD, MULT = mybir.AluOpType.add, mybir.AluOpType.mult
    NWt, NWh = cfg["NWt"], cfg["NWh"]
    WSt, WSh = cfg["WSt"], cfg["WSh"]
    BASEt, BASEh = cfg["BASEt"], cfg["BASEh"]
    Bt, Bh = NWt * BPW, NWh * BPW

    nc = bacc.Bacc(None, target_bir_lowering=False)
    # register the Sin bias constant
    _cb = nc.alloc_sbuf_tensor(f"const-float32-{HALF_PI}", [128, 1], f32)
    nc.gpsimd.memset(_cb.ap(), HALF_PI)
    nc.const_aps.aps[(f32, HALF_PI)] = _cb.ap()
    nc.all_engine_barrier()

    xt = nc.declare_dram_parameter("xt", [128, Bt * C], f32, isOutput=False)
    at = nc.declare_dram_parameter("at", [128, Bt * PP * SPAN], f16, isOutput=False)
    selt = nc.declare_dram_parameter("selt", [128, NWt * 16 * WSt], f8, isOutput=False)
    xh = nc.declare_dram_parameter("xh", [128, Bh * (C + SPAN)], f16, isOutput=False)
    selh = nc.declare_dram_parameter("selh", [128, NWh * 16 * WSh], f8, isOutput=False)
    u = nc.declare_dram_parameter("u", [64, GPC], f32, isOutput=True)

    NL1 = 3                 # persistent L1 PSUM tiles (2 banks each)

    with tile.TileContext(nc) as tc:
        import contextlib
        with contextlib.ExitStack() as ctx:
            xpool = ctx.enter_context(tc.tile_pool(name="x", bufs=3))
            cpool = ctx.enter_context(tc.tile_pool(name="cos", bufs=3))
            spool = ctx.enter_context(tc.tile_pool(name="stat", bufs=3))
            persist = ctx.enter_context(tc.tile_pool(name="persist", bufs=1))
            l1ps = ctx.enter_context(tc.tile_pool(name="l1ps", bufs=1, space="PSUM"))
            accps = ctx.enter_context(tc.tile_pool(name="accps", bufs=1, space="PSUM"))

            zs = persist.tile([128, 512], f16, tag="zs", name="zs")
            nc.gpsimd.memset(zs[:], 0.0)
            sel_t = persist.tile([128, NWt * 16 * WSt], f8, tag="selt")
            nc.sync.dma_start(out=sel_t[:], in_=selt[:])
            sel_h = persist.tile([128, NWh * 16 * WSh], f8, tag="selh")
            nc.sync.dma_start(out=sel_h[:], in_=selh[:])

            acc = accps.tile([64, GPC], f32, tag="acc", name="acc")
            l1t = [l1ps.tile([128, 16, 64], f32, tag=f"l1_{j}", name=f"l1_{j}")
                   for j in range(NL1)]
            hi = [persist.tile([128, 16, 64], f16, tag=f"hi_{j}", name=f"hi_{j}")
                  for j in range(NL1)]

            # zero acc bank and all L1 banks (kills stale PSUM NaN/Inf)
            nc.tensor.matmul(out=acc[:], lhsT=zs[:, 0:64], rhs=zs[:],
                             start=True, stop=False)
            for j in range(NL1):
                nc.tensor.matmul(out=l1t[j][:, 0:8, :], lhsT=zs[:, 0:128],
                                 rhs=zs[:], start=True, stop=True)
                nc.tensor.matmul(out=l1t[j][:, 8:16, :], lhsT=zs[:, 0:128],
                                 rhs=zs[:], start=True, stop=True)

            gidx = [0]      # global L1-group counter (round-robins l1t/hi)

            def emit_l2(j, sel_tile, W_SEL, BASE_k, goff):
                # one K=68 matmul per block pair, direct into acc
                for s in range(16):
                    c0 = (goff * 16 + s) * W_SEL
                    nc.tensor.matmul(
                        out=acc[0:C, int(BASE_k[s]):int(BASE_k[s]) + W_SEL],
                        lhsT=hi[j][0:68, s, 0:C],
                        rhs=sel_tile[0:68, c0:c0 + W_SEL],
                        start=False, stop=False)

            def torsion_window(k):
                j = gidx[0] % NL1
                x_t = xpool.tile([128, BPW * C], f32, tag="xt")
                nc.sync.dma_start(out=x_t[:], in_=xt[:, k * BPW * C:(k + 1) * BPW * C])
                a_t = spool.tile([128, BPW, PP * SPAN], f16, tag="at")
                nc.sync.dma_start(
                    out=a_t[:],
                    in_=at[:, k * BPW * PP * SPAN:(k + 1) * BPW * PP * SPAN])

                cos = {}
                for p in (1, 2, 4, 5):
                    cos[p] = cpool.tile([128, BPW * C], f16, tag=f"c{p}", name=f"cos{p}")
                    nc.scalar.activation(cos[p][:], x_t[:], SIN,
                                         bias=HALF_PI, scale=-float(p))
                h3 = cpool.tile([128, BPW * C], f16, tag="h3", name="h3")
                nc.vector.scalar_tensor_tensor(
                    out=h3[:], in0=cos[2][:], scalar=-0.5, in1=cos[1][:],
                    op0=ADD, op1=MULT)
                b6 = cpool.tile([128, BPW * C], f16, tag="b6", name="b6")
                nc.vector.tensor_mul(b6[:], h3[:], h3[:])
                basis = {1: cos[1], 2: cos[2], 3: h3, 4: cos[4], 5: cos[5], 6: b6}

                # bb-outer
                for bb in range(BPW):
                    a, s = bb % 2, bb // 2
                    for p in (1, 2, 3, 4, 5, 6):
                        nc.tensor.matmul(
                            out=l1t[j][64 * a:64 * a + 4, s, 0:C],
                            lhsT=a_t[:, bb, (p - 1) * SPAN:p * SPAN],
                            rhs=basis[p][:, C * bb:C * (bb + 1)],
                            start=(p == 1), stop=(p == 6))
                g = gidx[0]
                gidx[0] += 1
                def fin(j=j, k=k, g=g):
                    if g % 2 == 0:
                        nc.scalar.copy(hi[j][0:68, :, 0:C], l1t[j][0:68, :, 0:C])
                    else:
                        nc.vector.tensor_copy(hi[j][0:68, :, 0:C], l1t[j][0:68, :, 0:C])
                    emit_l2(j, sel_t, WSt, BASEt[k], k)
                return fin

            def harmonic_window(k):
                j = gidx[0] % NL1
                x_h = xpool.tile([128, BPW, C + SPAN], f16, tag="xh")
                nc.sync.dma_start(
                    out=x_h[:],
                    in_=xh[:, k * BPW * (C + SPAN):(k + 1) * BPW * (C + SPAN)])
                sq = cpool.tile([128, BPW, C], f16, tag="sq", name="sq")
                nc.vector.tensor_mul(sq[:], x_h[:, :, 0:C], x_h[:, :, 0:C])

                for bb in range(BPW):
                    a, s = bb % 2, bb // 2
                    nc.tensor.matmul(
                        out=l1t[j][64 * a:64 * a + 4, s, 0:C],
                        lhsT=x_h[:, bb, C:C + SPAN],
                        rhs=sq[:, bb, :],
                        start=True, stop=True)
                g = gidx[0]
                gidx[0] += 1
                def fin(j=j, k=k, g=g):
                    if g % 2 == 0:
                        nc.scalar.copy(hi[j][0:68, :, 0:C], l1t[j][0:68, :, 0:C])
                    else:
                        nc.vector.tensor_copy(hi[j][0:68, :, 0:C], l1t[j][0:68, :, 0:C])
                    emit_l2(j, sel_h, WSh, BASEh[k], k)
                return fin

            # interleave torsion and harmonic windows (ratio NWt:NWh),
            # delaying each window's copy+L2 by one window (engine queues
            # are in-order; the copy waits on PE and would head-of-line
            # block the next window's basis production)
            order = []
            ti, hj = 0, 0
            while ti < NWt or hj < NWh:
                for _ in range(2):
                    if hj < NWh:
                        order.append(("h", hj)); hj += 1
                if ti < NWt:
                    order.append(("t", ti)); ti += 1
            pending = None
            for typ, k in order:
                fin = torsion_window(k) if typ == "t" else harmonic_window(k)
                if pending is not None:
                    pending()
                pending = fin
            pending()

            # final stop matmul + read out
            nc.tensor.matmul(out=acc[:], lhsT=zs[:, 0:64], rhs=zs[:],
                             start=False, stop=True)
            outt = persist.tile([64, GPC], f32, tag="outt", name="outt")
            nc.vector.tensor_copy(outt[:], acc[:])
            nc.sync.dma_start(out=u[:], in_=outt[:])

    nc.finalize()
    return nc


# ----------------------------------------------------------------------------
# entry point
# ----------------------------------------------------------------------------

def kernel(**inputs) -> np.ndarray:
    global LAST_RESULTS
    in_maps, cfg, u_const = _prep_host(**inputs)
    nc = _build_nc(cfg)
    res = run_bass_kernel_spmd(
        nc, in_maps, list(range(NCORES)),
        trace=bool(os.environ.get("KERNEL_TRACE")))
    LAST_RESULTS = res
    out = np.empty((G, C), np.float32)
    for c in range(NCORES):
        out[c * GPC:(c + 1) * GPC] = res.results[c]["u"][0:C, :].T
    out += u_const[:, None].astype(np.float32)
    return out


# revision 3
# speedup vs baseline: 1.0339x; 1.0339x over previous
# Trainium2 Bass kernel for nn_EnergyInGraph (espaloma-style graph energy sum).
#
# Math:
#   u2 = 0.5*k2*(x2-eq2)^2            [N2, C]  harmonic bonds
#   u3 = 0.5*k3*(x3-eq3)^2            [N3, C]  harmonic angles
#   u4 = sum_p k4_p*(1+cos(p*x4))     [N4, C]  periodic torsions (phases=0)
#   out[g, c] = segment_sum(u2)+segment_sum(u3)+segment_sum(u4)   [G, C]
#
# Strategy (data-parallel over graphs, 8 cores, one SPMD program):
#   Host sorts each stream by gid, assigns 512 graphs per core, and packs
#   nodes into fixed gid-width windows of exactly 32 blocks (<=128 rows,
#   <=4 consecutive gids per block).  Window widths / sel offsets are tuned
#   to the actual data (max over cores) so the single program fits all
#   cores.
#   L1: per block, narrow-stationary matmuls ([128,4] one-hot / weighted
#   one-hot vs [128,50] basis) accumulate [4,50] strips into a 2-bank PSUM
#   tile (partition bases 0/64, 16 free slots -> 32 blocks).
#   L2: PSUM tile is copied to SBUF f16 (GPSIMD), then one K=68 transposed
#   matmul per block PAIR (lhsT = hi[0:68, s, 0:50] data, rhs = fp8 sel
#   one-hot [68, W_SEL]) accumulates directly into the final transposed
#   PSUM accumulator acc[50, 512] at window-fixed free offsets.
#   Torsion basis: ACT sin for cos(px), p in {1,2,4,5}; DVE fused ops for
#   h3 = cos(3x)/2 = (c2-0.5)*c1 and b6 = 1+cos(6x) = 8*h3^2 (weight
#   factors folded on host: w'_3 = 2*w3; p=6 const excluded from u_const).

import os
import numpy as np
import ml_dtypes

import concourse.bacc as bacc
import concourse.tile as tile
from concourse import mybir
from concourse.bass_utils import run_bass_kernel_spmd

FP8 = ml_dtypes.float8_e4m3
F16 = np.float16

N2, N3, N4, C, PP, G = 200_000, 400_000, 300_000, 50, 6, 4096
NCORES = 8
GPC = G // NCORES          # graphs per core (512)
SPAN = 4                   # gid slots per block
BPW = 32                   # blocks per window (= one 2-bank PSUM L1 tile)
HALF_PI = float(np.pi / 2.0)

LAST_RESULTS = None        # BassKernelResults of the most recent run


# ----------------------------------------------------------------------------
# host-side packing
# ----------------------------------------------------------------------------

def _pack_window(gids, lo, hi):
    """Greedy blocks (<=128 rows, gid span < SPAN) for one window's
    gid-sorted slice. gids are window-local-sorted global-local gids.
    Returns list of (start, nrows, g0)."""
    blocks = []
    n = len(gids)
    i = 0
    while i < n:
        g0 = int(gids[i])
        j_max = min(i + 128, n)
        j = i + int(np.searchsorted(gids[i:j_max], g0 + SPAN, side="left"))
        smax = int(gids[j - 1]) - g0           # max used slot index
        blocks.append((i, j - i, g0, smax))
        i = j
    return blocks


def _core_blocks(gid_sorted, W):
    """Split one core's sorted gid stream into ceil(512/W) windows and pack
    blocks. Returns list over windows of block lists."""
    nw = (GPC + W - 1) // W
    bounds = np.searchsorted(gid_sorted, np.arange(0, (nw + 1) * W, W))
    wins = []
    for k in range(nw):
        s, e = bounds[k], bounds[k + 1]
        wins.append([(s + b0, nr, g0, smax) for (b0, nr, g0, smax)
                     in _pack_window(gid_sorted[s:e], k * W, min((k + 1) * W, GPC))])
    return wins


def _choose_W(streams_gids, W_hint):
    """Largest W (block window gid width) such that every window on every
    core packs into <= BPW blocks."""
    for W in range(W_hint, 3, -1):
        ok = True
        for gs in streams_gids:
            for blks in _core_blocks(gs, W):
                if len(blks) > BPW:
                    ok = False
                    break
            if not ok:
                break
        if ok:
            return W
    raise RuntimeError("no feasible window width")


def _sel_layout(all_wins, W, W_SEL0):
    """Choose W_SEL and per-(window k, pair s) absolute output bases, uniform
    across cores, covering every real block pair. all_wins: [core][k] ->
    blocks list. Returns (W_SEL, BASE[k][s])."""
    nw = max(len(w) for w in all_wins)
    W_SEL = W_SEL0
    while W_SEL <= 64:
        # required coverage per (k, s): min/max slot gid over cores
        ok = True
        BASE = np.zeros((nw, BPW // 2), np.int64)
        for k in range(nw):
            for s in range(BPW // 2):
                lo_need, hi_need = None, None
                for wins in all_wins:
                    if k >= len(wins):
                        continue
                    for bb in (2 * s, 2 * s + 1):
                        if bb < len(wins[k]):
                            g0, smax = wins[k][bb][2], wins[k][bb][3]
                            lo_need = g0 if lo_need is None else min(lo_need, g0)
                            hi_need = (g0 + smax + 1) if hi_need is None else max(hi_need, g0 + smax + 1)
                if lo_need is None:
                    BASE[k][s] = min(k * W, GPC - W_SEL)
                    continue
                if hi_need - lo_need > W_SEL:
                    ok = False
                    break
                base = min(max(lo_need, 0), GPC - W_SEL)
                if hi_need > base + W_SEL:
                    base = min(hi_need - W_SEL, GPC - W_SEL)
                if lo_need < base:
                    ok = False
                    break
                BASE[k][s] = base
            if not ok:
                break
        if ok:
            return W_SEL, BASE
        W_SEL += 4
    raise RuntimeError("sel width explosion")


def _prep_stream(x_sorted, gid_sorted, w_sorted, W):
    """Per-core block packing + device arrays for one stream.
    x_sorted:[R,C] f32; w_sorted [R,K] weights or None (harmonic).
    Returns (wins, arrays dict building callback inputs)."""
    wins = _core_blocks(gid_sorted, W)
    return wins


def _fill_arrays(wins, x_sorted, gid_sorted, w_sorted, NW, W_SEL, BASE):
    """Build xpack [128, NW*BPW*C] f32 (or xh/oh merged for harmonic),
    at [128, NW*BPW*PP*SPAN] f16 (torsion) and sel [128, NW*16*W_SEL] fp8."""
    torsion = w_sorted is not None
    B = NW * BPW
    xp = np.zeros((B, 128, C), np.float32)
    at = np.zeros((B, 128, PP, SPAN), np.float32) if torsion else None
    oh = np.zeros((B, 128, SPAN), np.float32) if not torsion else None
    sel = np.zeros((NW, BPW // 2, 128, W_SEL), np.float32)
    R = len(gid_sorted)
    for k in range(len(wins)):
        for bb, (b0, nr, g0, smax) in enumerate(wins[k]):
            bi = k * BPW + bb
            idx = b0 + np.arange(nr)
            xp[bi, :nr] = x_sorted[idx]
            slots = gid_sorted[idx] - g0            # [nr] in [0, SPAN)
            if torsion:
                at[bi, np.arange(nr)[:, None], np.arange(PP)[None, :],
                   slots[:, None]] = w_sorted[idx]
            else:
                oh[bi, np.arange(nr), slots] = 1.0
            # sel: pair s = bb//2, partial row = 64*(bb%2) + i
            s = bb // 2
            base = BASE[k][s]
            for i in range(smax + 1):
                col = g0 + i - base
                assert 0 <= col < W_SEL, (k, bb, g0, base)
                sel[k, s, 64 * (bb % 2) + i, col] = 1.0
    out = {
        "sel": np.ascontiguousarray(
            sel.transpose(2, 0, 1, 3).reshape(128, NW * (BPW // 2) * W_SEL)
        ).astype(FP8),
    }
    if torsion:
        out["xt"] = np.ascontiguousarray(
            xp.transpose(1, 0, 2).reshape(128, B * C)).astype(F16)
        out["at"] = np.ascontiguousarray(
            at.transpose(1, 0, 2, 3).reshape(128, B * PP * SPAN)).astype(F16)
    else:
        xo = np.concatenate([xp, oh], axis=2)      # [B, 128, C+SPAN]
        out["xh"] = np.ascontiguousarray(
            xo.transpose(1, 0, 2).reshape(128, B * (C + SPAN))).astype(F16)
    return out


def _prep_host(x2, k2, eq2, gid2, x3, k3, eq3, gid3, x4, k4, phases4,
               periodicity4, gid4, n_graphs):
    G_ = int(n_graphs)
    assert G_ == G
    if np.count_nonzero(np.asarray(phases4)) != 0:
        raise NotImplementedError("nonzero torsion phases not supported")
    per = np.asarray(periodicity4)
    peri = np.rint(per).astype(np.int64)
    assert np.all((peri >= 1) & (peri <= PP))

    # torsion basis weights w4[n, p-1] = sum of k4 slots with periodicity p
    if np.array_equal(peri[0], np.arange(1, PP + 1)) and np.all(peri == peri[0]):
        w4 = np.asarray(k4, np.float32).copy()
    else:
        w4 = np.zeros((N4, PP), np.float32)
        np.add.at(w4, (np.arange(N4)[:, None], peri - 1), np.asarray(k4))

    # constant torsion term: sum_{p<=5} w_p per node -> per graph
    # (p=6 basis is 1+cos6 so its +1 is on device)
    const4 = np.asarray(w4[:, 0:5], np.float64).sum(1)
    u_const = np.bincount(np.asarray(gid4), weights=const4, minlength=G)
    # device basis for p=3 is cos(3x)/2 -> fold factor 2 into the weight;
    # p=6 basis is h3^2 = (1+cos 6x)/8 -> fold factor 8 into the weight
    w4[:, 2] *= 2.0
    w4[:, 5] *= 8.0

    # harmonic: fold scales into x on host: xh = sqrt(0.5 k) * (x - eq)
    s2 = np.sqrt(0.5 * np.asarray(k2, np.float32))
    s3 = np.sqrt(0.5 * np.asarray(k3, np.float32))
    xh = np.concatenate([(np.asarray(x2) - np.asarray(eq2)) * s2,
                         (np.asarray(x3) - np.asarray(eq3)) * s3],
                        0).astype(np.float32)
    gidh = np.concatenate([np.asarray(gid2), np.asarray(gid3)]).astype(np.int64)
    x4f = np.asarray(x4, np.float32)
    gid4l = np.asarray(gid4, np.int64)

    # sort streams by gid
    o = np.argsort(gidh, kind="stable")
    xh, gidh = xh[o], gidh[o]
    o = np.argsort(gid4l, kind="stable")
    x4s, gid4s, w4s = x4f[o], gid4l[o], w4[o]

    hsplit = np.searchsorted(gidh, np.arange(0, G + 1, GPC))
    tsplit = np.searchsorted(gid4s, np.arange(0, G + 1, GPC))
    t_g = [gid4s[tsplit[c]:tsplit[c + 1]] - c * GPC for c in range(NCORES)]
    h_g = [gidh[hsplit[c]:hsplit[c + 1]] - c * GPC for c in range(NCORES)]

    Wt = _choose_W(t_g, 60)
    Wh = _choose_W(h_g, 30)
    t_wins = [_core_blocks(g, Wt) for g in t_g]
    h_wins = [_core_blocks(g, Wh) for g in h_g]
    NWt = max(len(w) for w in t_wins)
    NWh = max(len(w) for w in h_wins)
    WSt, BASEt = _sel_layout(t_wins, Wt, 12)
    WSh, BASEh = _sel_layout(h_wins, Wh, 12)

    in_maps = []
    for c in range(NCORES):
        ts, te = tsplit[c], tsplit[c + 1]
        hs, he = hsplit[c], hsplit[c + 1]
        dt = _fill_arrays(t_wins[c], x4s[ts:te], t_g[c], w4s[ts:te],
                          NWt, WSt, BASEt)
        dh = _fill_arrays(h_wins[c], xh[hs:he], h_g[c], None,
                          NWh, WSh, BASEh)
        in_maps.append({"xt": dt["xt"], "at": dt["at"], "selt": dt["sel"],
                        "xh": dh["xh"], "selh": dh["sel"]})
    cfg = dict(NWt=NWt, NWh=NWh, WSt=WSt, WSh=WSh,
               BASEt=BASEt, BASEh=BASEh)
    return in_maps, cfg, u_const


# ----------------------------------------------------------------------------
# device kernel
# ----------------------------------------------------------------------------

def _build_nc(cfg):
    f32, f16, f8 = mybir.dt.float32, mybir.dt.float16, mybir.dt.float8e4
    SIN = mybir.ActivationFunctionType.Sin
    ADD, MULT = mybir.AluOpType.add, mybir.AluOpType.mult
    NWt, NWh = cfg["NWt"], cfg["NWh"]
    WSt, WSh = cfg["WSt"], cfg["WSh"]
    BASEt, BASEh = cfg["BASEt"], cfg["BASEh"]
    Bt, Bh = NWt * BPW, NWh * BPW

    nc = bacc.Bacc(None, target_bir_lowering=False)
    # register the Sin bias constant
    _cb = nc.alloc_sbuf_tensor(f"const-float32-{HALF_PI}", [128, 1], f32)
    nc.gpsimd.memset(_cb.ap(), HALF_PI)
    nc.const_aps.aps[(f32, HALF_PI)] = _cb.ap()
    _cb16 = nc.alloc_sbuf_tensor(f"const-float16-{HALF_PI}", [128, 1], f16)
    nc.gpsimd.memset(_cb16.ap(), HALF_PI)
    nc.const_aps.aps[(f16, HALF_PI)] = _cb16.ap()
    nc.all_engine_barrier()

    xt = nc.declare_dram_parameter("xt", [128, Bt * C], f16, isOutput=False)
    at = nc.declare_dram_parameter("at", [128, Bt * PP * SPAN], f16, isOutput=False)
    selt = nc.declare_dram_parameter("selt", [128, NWt * 16 * WSt], f8, isOutput=False)
    xh = nc.declare_dram_parameter("xh", [128, Bh * (C + SPAN)], f16, isOutput=False)
    selh = nc.declare_dram_parameter("selh", [128, NWh * 16 * WSh], f8, isOutput=False)
    u = nc.declare_dram_parameter("u", [64, GPC], f32, isOutput=True)

    NL1 = 3                 # persistent L1 PSUM tiles (2 banks each)

    with tile.TileContext(nc) as tc:
        import contextlib
        with contextlib.ExitStack() as ctx:
            xpool = ctx.enter_context(tc.tile_pool(name="x", bufs=3))
            cpool = ctx.enter_context(tc.tile_pool(name="cos", bufs=3))
            spool = ctx.enter_context(tc.tile_pool(name="stat", bufs=3))
            persist = ctx.enter_context(tc.tile_pool(name="persist", bufs=1))
            l1ps = ctx.enter_context(tc.tile_pool(name="l1ps", bufs=1, space="PSUM"))
            accps = ctx.enter_context(tc.tile_pool(name="accps", bufs=1, space="PSUM"))

            zs = persist.tile([128, 512], f16, tag="zs", name="zs")
            nc.gpsimd.memset(zs[:], 0.0)
            sel_t = persist.tile([128, NWt * 16 * WSt], f8, tag="selt")
            nc.sync.dma_start(out=sel_t[:], in_=selt[:])
            sel_h = persist.tile([128, NWh * 16 * WSh], f8, tag="selh")
            nc.sync.dma_start(out=sel_h[:], in_=selh[:])

            acc = accps.tile([64, GPC], f32, tag="acc", name="acc")
            l1t = [l1ps.tile([128, 16, 64], f32, tag=f"l1_{j}", name=f"l1_{j}")
                   for j in range(NL1)]
            hi = [persist.tile([128, 16, 64], f16, tag=f"hi_{j}", name=f"hi_{j}")
                  for j in range(NL1)]

            # zero acc bank and all L1 banks (kills stale PSUM NaN/Inf)
            nc.tensor.matmul(out=acc[:], lhsT=zs[:, 0:64], rhs=zs[:],
                             start=True, stop=False)
            for j in range(NL1):
                nc.tensor.matmul(out=l1t[j][:, 0:8, :], lhsT=zs[:, 0:128],
                                 rhs=zs[:], start=True, stop=True)
                nc.tensor.matmul(out=l1t[j][:, 8:16, :], lhsT=zs[:, 0:128],
                                 rhs=zs[:], start=True, stop=True)

            gidx = [0]      # global L1-group counter (round-robins l1t/hi)

            def emit_l2(j, sel_tile, W_SEL, BASE_k, goff):
                # one K=68 matmul per block pair, direct into acc
                for s in range(16):
                    c0 = (goff * 16 + s) * W_SEL
                    nc.tensor.matmul(
                        out=acc[0:C, int(BASE_k[s]):int(BASE_k[s]) + W_SEL],
                        lhsT=hi[j][0:68, s, 0:C],
                        rhs=sel_tile[0:68, c0:c0 + W_SEL],
                        start=False, stop=False)

            def torsion_window(k):
                j = gidx[0] % NL1
                x_t = xpool.tile([128, BPW * C], f16, tag="xt")
                nc.sync.dma_start(out=x_t[:], in_=xt[:, k * BPW * C:(k + 1) * BPW * C])
                a_t = spool.tile([128, BPW, PP * SPAN], f16, tag="at")
                nc.sync.dma_start(
                    out=a_t[:],
                    in_=at[:, k * BPW * PP * SPAN:(k + 1) * BPW * PP * SPAN])

                cos = {}
                for p in (1, 2, 4, 5):
                    cos[p] = cpool.tile([128, BPW * C], f16, tag=f"c{p}", name=f"cos{p}")
                    nc.scalar.activation(cos[p][:], x_t[:], SIN,
                                         bias=HALF_PI, scale=-float(p))
                h3 = cpool.tile([128, BPW * C], f16, tag="h3", name="h3")
                nc.vector.scalar_tensor_tensor(
                    out=h3[:], in0=cos[2][:], scalar=-0.5, in1=cos[1][:],
                    op0=ADD, op1=MULT)
                b6 = cpool.tile([128, BPW * C], f16, tag="b6", name="b6")
                nc.vector.tensor_mul(b6[:], h3[:], h3[:])
                basis = {1: cos[1], 2: cos[2], 3: h3, 4: cos[4], 5: cos[5], 6: b6}

                # bb-outer
                for bb in range(BPW):
                    a, s = bb % 2, bb // 2
                    for p in (1, 2, 3, 4, 5, 6):
                        nc.tensor.matmul(
                            out=l1t[j][64 * a:64 * a + 4, s, 0:C],
                            lhsT=a_t[:, bb, (p - 1) * SPAN:p * SPAN],
                            rhs=basis[p][:, C * bb:C * (bb + 1)],
                            start=(p == 1), stop=(p == 6))
                g = gidx[0]
                gidx[0] += 1
                def fin(j=j, k=k, g=g):
                    if g % 2 == 0:
                        nc.scalar.copy(hi[j][0:68, :, 0:C], l1t[j][0:68, :, 0:C])
                    else:
                        nc.vector.tensor_copy(hi[j][0:68, :, 0:C], l1t[j][0:68, :, 0:C])
                    emit_l2(j, sel_t, WSt, BASEt[k], k)
                return fin

            def harmonic_window(k):
                j = gidx[0] % NL1
                x_h = xpool.tile([128, BPW, C + SPAN], f16, tag="xh")
                nc.sync.dma_start(
                    out=x_h[:],
                    in_=xh[:, k * BPW * (C + SPAN):(k + 1) * BPW * (C + SPAN)])
                sq = cpool.tile([128, BPW, C], f16, tag="sq", name="sq")
                nc.vector.tensor_mul(sq[:], x_h[:, :, 0:C], x_h[:, :, 0:C])

                for bb in range(BPW):
                    a, s = bb % 2, bb // 2
                    nc.tensor.matmul(
                        out=l1t[j][64 * a:64 * a + 4, s, 0:C],
                        lhsT=x_h[:, bb, C:C + SPAN],
                        rhs=sq[:, bb, :],
                        start=True, stop=True)
                g = gidx[0]
                gidx[0] += 1
                def fin(j=j, k=k, g=g):
                    if g % 2 == 0:
                        nc.scalar.copy(hi[j][0:68, :, 0:C], l1t[j][0:68, :, 0:C])
                    else:
                        nc.vector.tensor_copy(hi[j][0:68, :, 0:C], l1t[j][0:68, :, 0:C])
                    emit_l2(j, sel_h, WSh, BASEh[k], k)
                return fin

            # interleave torsion and harmonic windows (ratio NWt:NWh),
            # delaying each window's copy+L2 by one window (engine queues
            # are in-order; the copy waits on PE and would head-of-line
            # block the next window's basis production)
            order = []
            ti, hj = 0, 0
            while ti < NWt or hj < NWh:
                for _ in range(2):
                    if hj < NWh:
                        order.append(("h", hj)); hj += 1
                if ti < NWt:
                    order.append(("t", ti)); ti += 1
            pending = None
            for typ, k in order:
                fin = torsion_window(k) if typ == "t" else harmonic_window(k)
                if pending is not None:
                    pending()
                pending = fin
            pending()

            # final stop matmul + read out
            nc.tensor.matmul(out=acc[:], lhsT=zs[:, 0:64], rhs=zs[:],
                             start=False, stop=True)
            outt = persist.tile([64, GPC], f32, tag="outt", name="outt")
            nc.vector.tensor_copy(outt[:], acc[:])
            nc.sync.dma_start(out=u[:], in_=outt[:])

    nc.finalize()
    return nc


# ----------------------------------------------------------------------------
# entry point
# ----------------------------------------------------------------------------

def kernel(**inputs) -> np.ndarray:
    global LAST_RESULTS
    in_maps, cfg, u_const = _prep_host(**inputs)
    nc = _build_nc(cfg)
    res = run_bass_kernel_spmd(
        nc, in_maps, list(range(NCORES)),
        trace=bool(os.environ.get("KERNEL_TRACE")))
    LAST_RESULTS = res
    out = np.empty((G, C), np.float32)
    for c in range(NCORES):
        out[c * GPC:(c + 1) * GPC] = res.results[c]["u"][0:C, :].T
    out += u_const[:, None].astype(np.float32)
    return out
